# revision 1
# baseline (speedup 1.0000x reference)
"""Trainium2 Bass kernel for 3-layer GCN (nn_MultiLayerGCN_48773648613817).

Strategy (8 NeuronCores, SPMD):
  - Nodes sharded across cores (12500/core, padded to 12544 = 98*128).
  - Per layer:  table = dis (.) (X @ W)  computed shard-local ([node,feat] rows),
    AllGather'd into a replicated DRAM table.
  - Edges partitioned by destination core, grouped into 128-node dest windows,
    padded to 128-edge chunks (layout shared across cores; per-core data).
  - Per chunk: indirect-DMA gather of 128 source rows (one row per
    partition), one-hot selection matrix S built on DVE via is_equal against an
    iota row, TensorE matmul S^T @ msg accumulated into the window's PSUM.
  - Window epilogue: out = relu(dis * psum + b); layers 1-2 transpose back to
    X^T for the next layer's matmul, layer 3 DMAs rows to the output.
  - Message tables: fp8-e4m3 for layers 0-1; layer 2 gathers a random 50%
    of its edges from an fp8 table and the rest from bf16 (per-edge class,
    each (window, class) run padded to 128 so every chunk reads one table).
    The random per-edge gathers are DMA-byte-bound on this part, so fp8
    cuts the dominant cost; full-fp8 layer 2 would breach the 2e-2 gate
    (measures ~0.023), the 75% mix predicts rel err 0.0173 (host sim is bit-faithful: predicted 0.01542 == measured 0.01542 at the 50% mix). The S^T @ msg
    matmul runs bf16 lhsT against fp8 rhs (PE allows mixed non-fp32 dtypes).

Self-loops are handled in the window epilogue via the hsb add (coefficient
dis^2 = 1/deg matches GCN's normalized self-loop exactly, since
msg = dis[src]*h[src] and the window epilogue multiplies by dis[dst]).
"""

import numpy as np

from concourse import bass, bacc, mybir, tile
from concourse.bass_utils import run_bass_kernel_spmd

N_NODES = 100000
N_LAYERS = 3
DIM = 128
N_CORES = 8
NSH = N_NODES // N_CORES          # 12500 real nodes per shard
P = 128
NWIN = 98                          # windows per shard
NSHP = NWIN * P                    # 12544 padded nodes per shard
N_TABLE = N_CORES * NSHP           # 100352 padded table rows

F32 = mybir.dt.float32
BF = mybir.dt.bfloat16
F8 = mybir.dt.float8e4
I32 = mybir.dt.int32

FP8_FRAC = (1.0, 1.0, 0.75)  # fraction of each layer's edges gathered fp8


def _prepare(x, edge_indices, W, b):
    """Host-side index preprocessing. Returns (in_maps, layout) where layout
    gives the compile-time chunk counts per (layer, window), shared by all
    cores."""
    x = np.asarray(x, dtype=np.float32)
    ei = np.asarray(edge_indices).astype(np.int64)
    W = np.asarray(W, dtype=np.float32)
    b = np.asarray(b, dtype=np.float32)

    import ml_dtypes
    BF16 = ml_dtypes.bfloat16
    # per-core constant inputs
    iota_row = np.broadcast_to(
        np.arange(P, dtype=np.float32)[None, :], (P, P)
    ).astype(BF16)
    ident = np.eye(P, dtype=np.float32)
    bb = b.reshape(1, N_LAYERS * DIM).copy()

    xts = []
    for c in range(N_CORES):
        xs = x[c * NSH : (c + 1) * NSH]                      # [12500, 128]
        xp = np.zeros((NSHP, DIM), dtype=np.float32)
        xp[:NSH] = xs
        xts.append(np.ascontiguousarray(xp.T))               # [128, 12544]

    degs = np.ones((N_CORES, N_LAYERS, P, NWIN), dtype=np.float32)
    per_core_edges = [[None] * N_LAYERS for _ in range(N_CORES)]
    n_chunks = np.zeros((N_LAYERS, NWIN), dtype=np.int64)

    cls_rng = np.random.default_rng(12345)
    chunk_cls = [None] * N_LAYERS
    win_cls_slots = [None] * N_LAYERS
    for l in range(N_LAYERS):
        row = ei[l, 0]
        col = ei[l, 1]
        frac = FP8_FRAC[l]
        mixed = 0.0 < frac < 1.0
        # per-edge precision class: 0 = fp8 table, 1 = bf16 table
        ecls = (
            (cls_rng.random(row.shape[0]) >= frac).astype(np.int32)
            if mixed
            else np.zeros(row.shape[0], np.int32)
        )
        deg = np.bincount(col, minlength=N_NODES).astype(np.float32) + 1.0
        src_pad = ((row // NSH) * NSHP + (row % NSH)).astype(np.int32)
        core_of = col // NSH
        lcol = (col % NSH).astype(np.int32)
        win = lcol // P
        dloc = (lcol % P).astype(np.float32)
        for c in range(N_CORES):
            m = core_of == c
            wc, dc, sc, kc = win[m], dloc[m], src_pad[m], ecls[m]
            # sort by (window, class) so each window is [fp8 run][bf16 run]
            order = np.argsort(wc * 2 + kc, kind="stable")
            wc, dc, sc, kc = wc[order], dc[order], sc[order], kc[order]
            cnt = np.bincount(wc * 2 + kc, minlength=NWIN * 2).reshape(NWIN, 2)
            per_core_edges[c][l] = (cnt, dc, sc)
            dlp = np.ones(NSHP, dtype=np.float32)
            dlp[:NSH] = deg[c * NSH : (c + 1) * NSH]
            degs[c, l] = dlp.reshape(NWIN, P).T
        cnts = np.stack([per_core_edges[c][l][0] for c in range(N_CORES)])
        if mixed:
            # each (window, class) run padded to 128 so every chunk maps to
            # exactly one (window, table)
            mxc = np.maximum(cnts.max(axis=0), (1, 0))
            wcs = (mxc + P - 1) // P * P          # [NWIN, 2]
            win_cls_slots[l] = wcs
            n_chunks[l] = wcs.sum(axis=1)
            cc = []
            for w in range(NWIN):
                cc += [0] * (int(wcs[w, 0]) // P) + [1] * (int(wcs[w, 1]) // P)
            chunk_cls[l] = cc
        else:
            mx = np.maximum(cnts.sum(axis=2).max(axis=0), 1)
            n_chunks[l] = (mx + 63) // 64 * 64

    slots_layer = n_chunks.sum(axis=1)
    t_layer = (slots_layer + (P - 1)) // P         # chunks per layer
    tmax = int(t_layer.max())

    srcs_all = np.zeros((N_CORES, N_LAYERS, P, tmax), dtype=np.int32)
    dloc_all = np.full((N_CORES, N_LAYERS, P, tmax), -1.0, dtype=np.float32)
    for l in range(N_LAYERS):
        mixed = win_cls_slots[l] is not None
        for c in range(N_CORES):
            cnt, dc, sc = per_core_edges[c][l]
            off = np.concatenate([[0], np.cumsum(cnt.ravel())[:-1]]).reshape(
                NWIN, 2
            )
            tl = int(t_layer[l])
            s_arr = np.zeros((tl * P,), dtype=np.int32)
            d_arr = np.full((tl * P,), -1.0, dtype=np.float32)
            pos = 0
            for w in range(NWIN):
                if mixed:
                    for j in (0, 1):
                        nreal = int(cnt[w, j])
                        o0 = int(off[w, j])
                        s_arr[pos : pos + nreal] = sc[o0 : o0 + nreal]
                        d_arr[pos : pos + nreal] = dc[o0 : o0 + nreal]
                        pos += int(win_cls_slots[l][w, j])
                else:
                    nreal = int(cnt[w].sum())
                    o0 = int(off[w, 0])
                    s_arr[pos : pos + nreal] = sc[o0 : o0 + nreal]
                    d_arr[pos : pos + nreal] = dc[o0 : o0 + nreal]
                    pos += int(n_chunks[l, w])
            srcs_all[c, l, :, :tl] = s_arr.reshape(tl, P).T
            dloc_all[c, l, :, :tl] = d_arr.reshape(tl, P).T

    in_maps = []
    for c in range(N_CORES):
        in_maps.append(
            {
                "xt": xts[c],
                "wmat": W,
                "bb": bb,
                "iota": iota_row,
                "ident": ident,
                "degs": degs[c],
                "srcs": srcs_all[c],
                "dlocs": dloc_all[c].astype(BF16),
            }
        )
    layout = (n_chunks, t_layer, tmax, chunk_cls)
    return in_maps, layout


def _build(layout, skip_collective=False, msg_bufs=16):
    n_chunks, t_layer, tmax = layout[:3]
    chunk_cls = layout[3] if len(layout) > 3 else [None] * N_LAYERS
    nc = bacc.Bacc(
        "TRN2", target_bir_lowering=False, debug=False, num_devices=N_CORES
    )
    xt_in = nc.dram_tensor("xt", [P, NSHP], F32, kind="ExternalInput").ap()
    w_in = nc.dram_tensor("wmat", [N_LAYERS, DIM, DIM], F32, kind="ExternalInput").ap()
    b_in = nc.dram_tensor("bb", [1, N_LAYERS * DIM], F32, kind="ExternalInput").ap()
    iota_in = nc.dram_tensor("iota", [P, P], BF, kind="ExternalInput").ap()
    id_in = nc.dram_tensor("ident", [P, P], F32, kind="ExternalInput").ap()
    deg_in = nc.dram_tensor("degs", [N_LAYERS, P, NWIN], F32, kind="ExternalInput").ap()
    srcs_in = nc.dram_tensor("srcs", [N_LAYERS, P, tmax], I32, kind="ExternalInput").ap()
    dloc_in = nc.dram_tensor("dlocs", [N_LAYERS, P, tmax], BF, kind="ExternalInput").ap()
    out_ap = nc.dram_tensor("out", [NSHP, DIM], F32, kind="ExternalOutput").ap()

    hloc16 = nc.dram_tensor("hloc16", [NSHP, DIM], BF).ap()
    table16 = nc.dram_tensor("table16", [N_TABLE, DIM], BF, addr_space="Shared").ap()
    hloc8 = nc.dram_tensor("hloc8", [NSHP, DIM], F8).ap()
    table8 = nc.dram_tensor("table8", [N_TABLE, DIM], F8, addr_space="Shared").ap()

    with tile.TileContext(nc) as tc:
        with (
            tc.tile_pool(name="const", bufs=1) as constp,
            tc.tile_pool(name="xt", bufs=1) as xtp,
            tc.tile_pool(name="edges", bufs=2) as edgep,
            tc.tile_pool(name="msg", bufs=msg_bufs) as msgp,
            tc.tile_pool(name="sel", bufs=2) as selp,
            tc.tile_pool(name="hsb", bufs=1) as hsbp,
            tc.tile_pool(name="tr", bufs=3) as trp,
            tc.tile_pool(name="ph", bufs=2, space="PSUM") as php,
            tc.tile_pool(name="pw", bufs=2, space="PSUM") as pwp,
            tc.tile_pool(name="pt", bufs=2, space="PSUM") as ptp,
            tc.tile_pool(name="pb", bufs=1, space="PSUM") as pbp,
        ):
            # constants
            iota_sb = constp.tile([P, P], BF)
            nc.sync.dma_start(out=iota_sb[:], in_=iota_in[:])
            ident_sb = constp.tile([P, P], F32)
            nc.sync.dma_start(out=ident_sb[:], in_=id_in[:])
            w_sb = constp.tile([P, N_LAYERS * DIM], F32)
            for l in range(N_LAYERS):
                nc.sync.dma_start(
                    out=w_sb[:, l * DIM : (l + 1) * DIM], in_=w_in[l]
                )
            brow_sb = constp.tile([1, N_LAYERS * DIM], F32)
            nc.sync.dma_start(out=brow_sb[:], in_=b_in[:])
            ones_row = constp.tile([1, P], F32)
            nc.vector.memset(ones_row[:], 1.0)

            xt_sb = xtp.tile([P, NSHP], F32)
            nc.sync.dma_start(out=xt_sb[:], in_=xt_in[:])

            state = {}

            def epilogue(l, w, pw):
                dis_sb = state["dis_sb"]
                bbc_sb = state["bbc_sb"]
                hsb = state["hsb"]
                t0g = trp.tile([P, P], F32, tag="t0g")
                nc.vector.tensor_tensor(
                    out=t0g[:],
                    in0=pw[:],
                    in1=hsb[:, w * P : (w + 1) * P],
                    op=mybir.AluOpType.add,
                )
                t1 = trp.tile([P, P], F32, tag="t1")
                nc.vector.tensor_scalar(
                    out=t1[:],
                    in0=t0g[:],
                    scalar1=dis_sb[:, w : w + 1],
                    scalar2=None,
                    op0=mybir.AluOpType.mult,
                )
                nc.vector.tensor_tensor(
                    out=t1[:], in0=t1[:], in1=bbc_sb[:], op=mybir.AluOpType.add
                )
                t2 = trp.tile([P, P], F32, tag="t2")
                nc.vector.tensor_scalar(
                    out=t2[:],
                    in0=t1[:],
                    scalar1=0.0,
                    scalar2=None,
                    op0=mybir.AluOpType.max,
                )
                if l < N_LAYERS - 1:
                    pt = ptp.tile([P, P], F32, space="PSUM", tag="pt")
                    nc.tensor.transpose(out=pt[:], in_=t2[:], identity=ident_sb[:])
                    nc.vector.tensor_copy(
                        out=state["xt_sb"][:, w * P : (w + 1) * P], in_=pt[:]
                    )
                else:
                    nc.sync.dma_start(
                        out=out_ap[w * P : (w + 1) * P, :], in_=t2[:]
                    )

            state["xt_sb"] = xt_sb
            for l in range(N_LAYERS):
                tl = int(t_layer[l])
                # --- normalization coefficients ---
                deg_sb = trp.tile([P, NWIN], F32, tag="deg")
                nc.sync.dma_start(out=deg_sb[:], in_=deg_in[l])
                dis_sb = trp.tile([P, NWIN], F32, tag="dis")
                nc.vector.reciprocal(dis_sb[:], deg_sb[:])
                nc.scalar.activation(
                    dis_sb[:], dis_sb[:], mybir.ActivationFunctionType.Sqrt
                )

                # --- b broadcast tile: ones_row^T (x) b_row ---
                pb = pbp.tile([P, P], F32, space="PSUM", tag="pb")
                nc.tensor.matmul(
                    out=pb[:],
                    lhsT=ones_row[:],
                    rhs=brow_sb[:, l * DIM : (l + 1) * DIM],
                    start=True,
                    stop=True,
                )
                bbc_sb = trp.tile([P, P], F32, tag="bbc")
                nc.vector.tensor_copy(out=bbc_sb[:], in_=pb[:])
                state["dis_sb"] = dis_sb
                state["bbc_sb"] = bbc_sb

                # --- H stage: table_local = dis * (X @ W) ---
                frac = FP8_FRAC[l]
                has8 = frac > 0.0
                has16 = frac < 1.0
                mdt = F8 if (has8 and not has16) else BF
                hsb = hsbp.tile([P, NWIN * P], mdt, tag="hsb8" if mdt is F8 else "hsb")
                hsb8 = (
                    hsbp.tile([P, NWIN * P], F8, tag="hsb8", name="hsb8x")
                    if (has8 and has16)
                    else None
                )
                state["hsb"] = hsb
                for w in range(NWIN):
                    ph = php.tile([P, P], F32, space="PSUM", tag="ph")
                    nc.tensor.matmul(
                        out=ph[:],
                        lhsT=xt_sb[:, w * P : (w + 1) * P],
                        rhs=w_sb[:, l * DIM : (l + 1) * DIM],
                        start=True,
                        stop=True,
                    )
                    nc.vector.tensor_scalar(
                        out=hsb[:, w * P : (w + 1) * P],
                        in0=ph[:],
                        scalar1=dis_sb[:, w : w + 1],
                        scalar2=None,
                        op0=mybir.AluOpType.mult,
                    )
                    if hsb8 is not None:
                        nc.scalar.activation(
                            out=hsb8[:, w * P : (w + 1) * P],
                            in_=ph[:],
                            func=mybir.ActivationFunctionType.Copy,
                            scale=dis_sb[:, w : w + 1],
                        )
                pairs = []
                if has8:
                    pairs.append((hsb8 if hsb8 is not None else hsb, hloc8, table8))
                if has16:
                    pairs.append((hsb, hloc16, table16))
                for hs_t, hl_t, tb_t in pairs:
                    nc.sync.dma_start(
                        out=hl_t[:].rearrange("(w p) f -> p w f", p=P),
                        in_=hs_t[:].rearrange("p (w f) -> p w f", f=DIM),
                    )
                    if skip_collective:
                        nc.sync.dma_start(out=tb_t[:NSHP, :], in_=hl_t[:])
                    else:
                        nc.gpsimd.collective_compute(
                            "AllGather",
                            mybir.AluOpType.bypass,
                            replica_groups=[list(range(N_CORES))],
                            ins=[hl_t[:]],
                            outs=[tb_t[:]],
                        )

                # --- edge metadata for this layer ---
                srcs_sb = edgep.tile([P, tl], I32, tag="srcs")
                nc.sync.dma_start(out=srcs_sb[:], in_=srcs_in[l, :, :tl])
                dloc_sb = edgep.tile([P, tl], BF, tag="dlocs")
                nc.sync.dma_start(out=dloc_sb[:], in_=dloc_in[l, :, :tl])

                # --- scatter stage (slot-stream chunking) ---
                # window w owns slot range [wstart[w], wstart[w+1]); chunks are
                # 128-slot groups; a chunk may span window boundaries and is
                # consumed by per-window matmuls over partition subranges.
                slots = [int(n_chunks[l, w]) for w in range(NWIN)]
                wstart = [0]
                for w in range(NWIN):
                    wstart.append(wstart[-1] + slots[w])
                total_slots = wstart[-1]
                SB = 8  # chunks per S-build batch
                pw = None
                first = True
                wptr = 0
                s_base = 0
                for t in range(tl):
                    if t % SB == 0:
                        nb = min(SB, tl - t)
                        s_sb = selp.tile([P, SB * P], BF, tag="sel")
                        nc.vector.tensor_tensor(
                            out=s_sb[:, : nb * P].rearrange(
                                "p (k j) -> p k j", k=nb
                            ),
                            in0=dloc_sb[:, t : t + nb]
                            .unsqueeze(2)
                            .to_broadcast([P, nb, P]),
                            in1=iota_sb[:]
                            .unsqueeze(1)
                            .to_broadcast([P, nb, P]),
                            op=mybir.AluOpType.is_equal,
                        )
                        s_base = t
                    cls8 = has8 and (chunk_cls[l] is None or chunk_cls[l][t] == 0)
                    msg = msgp.tile([P, P], F8 if cls8 else BF,
                                    tag="msg8" if cls8 else "msg")
                    nc.gpsimd.indirect_dma_start(
                        out=msg[:],
                        out_offset=None,
                        in_=(table8 if cls8 else table16)[:],
                        in_offset=bass.IndirectOffsetOnAxis(
                            ap=srcs_sb[:, t : t + 1], axis=0
                        ),
                    )
                    scol = (t - s_base) * P
                    lo = t * P
                    hi = min(lo + P, total_slots)
                    a = 0
                    while lo + a < hi:
                        while wstart[wptr + 1] <= lo + a:
                            wptr += 1
                        w = wptr
                        bnd = min(hi, wstart[w + 1]) - lo
                        if pw is None:
                            pw = pwp.tile([P, P], F32, space="PSUM", tag="pw")
                            first = True
                        is_last = lo + bnd == wstart[w + 1]
                        nc.tensor.matmul(
                            out=pw[:],
                            lhsT=s_sb[a:bnd, scol : scol + P],
                            rhs=msg[a:bnd, :],
                            start=first,
                            stop=is_last,
                        )
                        first = False
                        if is_last:
                            epilogue(l, w, pw)
                            pw = None
                        a = bnd

    nc.compile()
    return nc


def build_all(x, edge_indices, W, b):
    in_maps, layout = _prepare(x, edge_indices, W, b)
    nc = _build(layout)
    return nc, in_maps


def kernel(x, edge_indices, W, b):
    nc, in_maps = build_all(x, edge_indices, W, b)
    last_err = None
    for _ in range(3):  # retry transient NRT/axon device faults
        try:
            res = run_bass_kernel_spmd(nc, in_maps, list(range(N_CORES)))
            break
        except Exception as e:  # noqa: BLE001
            last_err = e
            import time as _time

            _time.sleep(5.0)
    else:
        raise last_err
    out = np.concatenate(
        [res.results[c]["out"][:NSH] for c in range(N_CORES)], axis=0
    )
    return out.astype(np.float32)



# revision 11
# speedup vs baseline: 8.8683x; 8.8683x over previous
"""Trainium2 Bass kernel for 3-layer GCN (nn_MultiLayerGCN_48773648613817).

Strategy (8 NeuronCores, SPMD):
  - Nodes sharded across cores (12500/core, padded to 12544 = 98*128).
  - Per layer: hsb = dis (.) (X @ W) computed shard-local, AllGather'd into a
    replicated bf16 DRAM table (100352 rows).
  - Edges partitioned by destination core, grouped into 128-node dest windows.
    Within each window, edges are sorted into 4 source-segment classes
    (segment = src_row // 25088, so the int16 gather index fits); each
    (window, class) run is padded to a multiple of 128 (shared layout across
    cores via max-over-cores counts).
  - Gathers use batched dma_gather (InstDMAGatherAnt): K=16 chunks (2048 rows
    of 256B) per instruction, one SWDGE queue per class (num_swdge_queues=4),
    single_packet=False (>64-descriptor packets hang the SDMA engine).
  - Per chunk: one-hot S built on DVE via is_equal (16 chunks per op),
    TensorE matmul S^T @ msg accumulated into the window's PSUM.
  - Bias is folded into PSUM as the rank-1 sqrt(deg) (x) b matmul; the
    self-loop term rides PSUM via an identity matmul against hsb.
  - Window epilogue: out = relu(dis * psum); layers 0-1 transpose back into
    X^T for the next layer's H matmul, layer 2 DMAs rows to the output.
"""

import numpy as np

from concourse import bass, bacc, mybir, tile, library_config
from concourse.bass_utils import run_bass_kernel_spmd

N_NODES = 100000
N_LAYERS = 3
DIM = 128
N_CORES = 8
NSH = N_NODES // N_CORES          # 12500 real nodes per shard
P = 128
NWIN = 98                          # windows per shard
NSHP = NWIN * P                    # 12544 padded nodes per shard
N_TABLE = N_CORES * NSHP           # 100352 padded table rows
NSEG = 4
SEGR = N_TABLE // NSEG             # 25088 rows per gather segment

K = 16                             # chunks per dma_gather
SB = 16                            # chunks per S-build op

F32 = mybir.dt.float32
BF = mybir.dt.bfloat16
I16 = mybir.dt.int16


def _prepare(x, edge_indices, W, b):
    """Host-side preprocessing. Returns (in_maps, layout)."""
    x = np.asarray(x, dtype=np.float32)
    ei = np.asarray(edge_indices).astype(np.int64)
    W = np.asarray(W, dtype=np.float32)
    b = np.asarray(b, dtype=np.float32)

    import ml_dtypes
    BF16 = ml_dtypes.bfloat16

    iota_row = np.broadcast_to(
        np.arange(P, dtype=np.float32)[None, :], (P, P)
    ).astype(BF16)
    ident_bf = np.eye(P, dtype=np.float32).astype(BF16)
    bb = b.reshape(1, N_LAYERS * DIM).copy()

    xts = []
    for c in range(N_CORES):
        xp = np.zeros((NSHP, DIM), dtype=np.float32)
        xp[:NSH] = x[c * NSH : (c + 1) * NSH]
        xts.append(np.ascontiguousarray(xp.T))               # [128, 12544]

    degs = np.ones((N_CORES, N_LAYERS, P, NWIN), dtype=np.float32)
    # per (core, layer): edge arrays sorted by (window, class)
    per_core = [[None] * N_LAYERS for _ in range(N_CORES)]
    cnts = np.zeros((N_CORES, N_LAYERS, NWIN, NSEG), dtype=np.int64)

    for l in range(N_LAYERS):
        row = ei[l, 0]
        col = ei[l, 1]
        deg = np.bincount(col, minlength=N_NODES).astype(np.float32) + 1.0
        src_pad = ((row // NSH) * NSHP + (row % NSH)).astype(np.int64)
        seg = (src_pad // SEGR).astype(np.int64)
        idx16 = (src_pad % SEGR).astype(np.int16)
        core_of = col // NSH
        lcol = (col % NSH).astype(np.int64)
        win = lcol // P
        dloc = (lcol % P).astype(np.float32)
        for c in range(N_CORES):
            m = core_of == c
            wc, sc, ic, dc = win[m], seg[m], idx16[m], dloc[m]
            key = wc * NSEG + sc
            order = np.argsort(key, kind="stable")
            wc, sc, ic, dc = wc[order], sc[order], ic[order], dc[order]
            cnt = np.bincount(
                wc * NSEG + sc, minlength=NWIN * NSEG
            ).reshape(NWIN, NSEG)
            per_core[c][l] = (cnt, ic, dc)
            cnts[c, l] = cnt
            dlp = np.ones(NSHP, dtype=np.float32)
            dlp[:NSH] = deg[c * NSH : (c + 1) * NSH]
            degs[c, l] = dlp.reshape(NWIN, P).T

    # shared layout: chunks per (layer, window, class)
    mx = cnts.max(axis=0)                                    # [L, NWIN, NSEG]
    m_chunks = (mx + P - 1) // P                             # chunk counts
    tl_layer = m_chunks.sum(axis=(1, 2))                     # [L]
    tlc_layer = m_chunks.sum(axis=1)                         # [L, NSEG] chunks/class
    tmax = int(tl_layer.max())

    # per-core streams
    dloc_all = np.full((N_CORES, N_LAYERS, P, tmax), -1.0, dtype=np.float32)
    idx_all = np.zeros((N_CORES, N_LAYERS, P, tmax * 8), dtype=np.int16)
    for l in range(N_LAYERS):
        tl = int(tl_layer[l])
        tlc = tlc_layer[l]
        coff = np.concatenate([[0], np.cumsum(tlc)[:-1]])    # class chunk offsets
        for c in range(N_CORES):
            cnt, ic, dc = per_core[c][l]
            off = np.concatenate([[0], np.cumsum(cnt.ravel())[:-1]]).reshape(
                NWIN, NSEG
            )
            # slot stream (dloc) and class streams (idx)
            d_arr = np.full((tl * P,), -1.0, dtype=np.float32)
            i_arr = [np.zeros((int(tlc[s]) * P,), dtype=np.int16)
                     for s in range(NSEG)]
            cpos = [0] * NSEG
            spos = 0
            for w in range(NWIN):
                for s in range(NSEG):
                    mws = int(m_chunks[l, w, s])
                    if mws == 0:
                        continue
                    nreal = int(cnt[w, s])
                    o0 = int(off[w, s])
                    d_arr[spos : spos + nreal] = dc[o0 : o0 + nreal]
                    i_arr[s][cpos[s] * P : cpos[s] * P + nreal] = (
                        ic[o0 : o0 + nreal]
                    )
                    spos += mws * P
                    cpos[s] += mws
            dloc_all[c, l, :, :tl] = d_arr.reshape(tl, P).T
            # wrap idx streams: linear j -> [j%16, j//16], replicated x8
            colpos = 0
            for s in range(NSEG):
                n = i_arr[s].shape[0]
                if n == 0:
                    continue
                wrapped = i_arr[s].reshape(n // 16, 16).T    # [16, n//16]
                idx_all[c, l, :, colpos : colpos + n // 16] = np.tile(
                    wrapped, (8, 1)
                )
                colpos += n // 16

    in_maps = []
    for c in range(N_CORES):
        in_maps.append(
            {
                "xt": xts[c],
                "wmat": W,
                "bb": bb,
                "bcolT": np.ascontiguousarray(b.T),
                "iota": iota_row,
                "identb": ident_bf,
                "degs": degs[c],
                "idxs": idx_all[c],
                "dlocs": dloc_all[c].astype(BF16),
            }
        )
    layout = (m_chunks, tl_layer, tlc_layer, tmax)
    return in_maps, layout


def _build(layout, msg_bufs=3):
    m_chunks, tl_layer, tlc_layer, tmax = layout
    nc = bacc.Bacc(
        "TRN2",
        target_bir_lowering=False,
        debug=False,
        num_devices=N_CORES,
        num_swdge_queues=4,
    )
    xt_in = nc.dram_tensor("xt", [P, NSHP], F32, kind="ExternalInput").ap()
    w_in = nc.dram_tensor("wmat", [N_LAYERS, DIM, DIM], F32, kind="ExternalInput").ap()
    b_in = nc.dram_tensor("bb", [1, N_LAYERS * DIM], F32, kind="ExternalInput").ap()
    bcol_in = nc.dram_tensor("bcolT", [DIM, N_LAYERS], F32, kind="ExternalInput").ap()
    iota_in = nc.dram_tensor("iota", [P, P], BF, kind="ExternalInput").ap()
    identb_in = nc.dram_tensor("identb", [P, P], BF, kind="ExternalInput").ap()
    deg_in = nc.dram_tensor("degs", [N_LAYERS, P, NWIN], F32, kind="ExternalInput").ap()
    idxs_in = nc.dram_tensor("idxs", [N_LAYERS, P, tmax * 8], I16, kind="ExternalInput").ap()
    dloc_in = nc.dram_tensor("dlocs", [N_LAYERS, P, tmax], BF, kind="ExternalInput").ap()
    out_ap = nc.dram_tensor("out", [NSHP, DIM], F32, kind="ExternalOutput").ap()

    hloc16 = nc.dram_tensor("hloc16", [NSHP, DIM], BF).ap()
    table16 = nc.dram_tensor("table16", [N_TABLE, DIM], BF, addr_space="Shared").ap()

    with tile.TileContext(nc) as tc:
        with (
            tc.tile_pool(name="const", bufs=1) as constp,
            tc.tile_pool(name="xt", bufs=1) as xtp,
            tc.tile_pool(name="edges", bufs=1) as edgep,
            tc.tile_pool(name="dl", bufs=2) as dlp,
            tc.tile_pool(name="msg", bufs=msg_bufs) as msgp,
            tc.tile_pool(name="sel", bufs=2) as selp,
            tc.tile_pool(name="hsb", bufs=1) as hsbp,
            tc.tile_pool(name="tr", bufs=3) as trp,
            tc.tile_pool(name="ph", bufs=2, space="PSUM") as php,
            tc.tile_pool(name="pw", bufs=2, space="PSUM") as pwp,
            tc.tile_pool(name="pt", bufs=2, space="PSUM") as ptp,
            tc.tile_pool(name="pb", bufs=1, space="PSUM") as pbp,
        ):
            nc.gpsimd.load_library(library_config.mlp)

            iota_sb = constp.tile([P, P], BF)
            nc.sync.dma_start(out=iota_sb[:], in_=iota_in[:])
            identb_sb = constp.tile([P, P], BF)
            nc.sync.dma_start(out=identb_sb[:], in_=identb_in[:])
            w_sb = constp.tile([P, N_LAYERS * DIM], F32)
            for l in range(N_LAYERS):
                nc.sync.dma_start(out=w_sb[:, l * DIM : (l + 1) * DIM], in_=w_in[l])
            brow_sb = constp.tile([1, N_LAYERS * DIM], F32)
            nc.sync.dma_start(out=brow_sb[:], in_=b_in[:])
            bcol_sb = constp.tile([DIM, N_LAYERS], F32)
            nc.sync.dma_start(out=bcol_sb[:], in_=bcol_in[:])
            ones_row = constp.tile([1, P], F32)
            nc.vector.memset(ones_row[:], 1.0)

            xt_sb = xtp.tile([P, NSHP], F32)
            nc.sync.dma_start(out=xt_sb[:], in_=xt_in[:])

            for l in range(N_LAYERS):
                tl = int(tl_layer[l])
                tlc = [int(t) for t in tlc_layer[l]]
                coff = [0] * NSEG
                for s in range(1, NSEG):
                    coff[s] = coff[s - 1] + tlc[s - 1]

                # ---- normalization: dis = 1/sqrt(deg), [dest_p, w] ---------
                deg_sb = trp.tile([P, NWIN], F32, tag="deg")
                nc.sync.dma_start(out=deg_sb[:], in_=deg_in[l])
                dis_sb = trp.tile([P, NWIN], F32, tag="dis")
                nc.vector.reciprocal(dis_sb[:], deg_sb[:])
                nc.scalar.activation(
                    dis_sb[:], dis_sb[:], mybir.ActivationFunctionType.Sqrt
                )

                if l == N_LAYERS - 1:
                    # b replicated per partition (for the untransposed output)
                    pb2 = pbp.tile([P, P], F32, space="PSUM", tag="pb")
                    nc.tensor.matmul(
                        out=pb2[:],
                        lhsT=ones_row[:],
                        rhs=brow_sb[:, l * DIM : (l + 1) * DIM],
                        start=True,
                        stop=True,
                    )
                    bbc_sb = trp.tile([P, P], F32, tag="bbc")
                    nc.vector.tensor_copy(out=bbc_sb[:], in_=pb2[:])

                # ---- edge metadata ----------------------------------------
                idxs_sb = edgep.tile([P, tmax * 8], I16, tag="idxs")
                nc.sync.dma_start(out=idxs_sb[:, : tl * 8], in_=idxs_in[l, :, : tl * 8])
                dloc_sb = dlp.tile([P, tmax], BF, tag="dlocs")
                nc.sync.dma_start(out=dloc_sb[:, :tl], in_=dloc_in[l, :, :tl])

                # ---- H stage: hsb = dis (.) (X @ W), AllGather ------------
                hsb = hsbp.tile([P, NWIN * P], BF, tag="hsb")
                for w in range(NWIN):
                    ph = php.tile([P, P], F32, space="PSUM", tag="ph")
                    nc.tensor.matmul(
                        out=ph[:],
                        lhsT=xt_sb[:, w * P : (w + 1) * P],
                        rhs=w_sb[:, l * DIM : (l + 1) * DIM],
                        start=True,
                        stop=True,
                    )
                    nc.vector.tensor_scalar(
                        out=hsb[:, w * P : (w + 1) * P],
                        in0=ph[:],
                        scalar1=dis_sb[:, w : w + 1],
                        scalar2=None,
                        op0=mybir.AluOpType.mult,
                    )
                nc.sync.dma_start(
                    out=hloc16[:].rearrange("(w p) f -> p w f", p=P),
                    in_=hsb[:].rearrange("p (w f) -> p w f", f=DIM),
                )
                nc.gpsimd.collective_compute(
                    "AllGather",
                    mybir.AluOpType.bypass,
                    replica_groups=[list(range(N_CORES))],
                    ins=[hloc16[:]],
                    outs=[table16[:]],
                )

                # ---- scatter stage ----------------------------------------
                cpos = [0] * NSEG
                cur_msg = [None] * NSEG
                s_sb = None
                t = 0
                for w in range(NWIN):
                    wtot = int(m_chunks[l, w].sum())
                    pw = pwp.tile([P, P], F32, space="PSUM", tag="pw")
                    nc.tensor.matmul(
                        out=pw[:],
                        lhsT=identb_sb[:],
                        rhs=hsb[:, w * P : (w + 1) * P],
                        start=True,
                        stop=(wtot == 0),
                    )
                    done = 0
                    for s in range(NSEG):
                        for _ in range(int(m_chunks[l, w, s])):
                            if t % SB == 0:
                                nb = min(SB, tl - t)
                                s_sb = selp.tile([P, SB * P], BF, tag="sel")
                                nc.vector.tensor_tensor(
                                    out=s_sb[:, : nb * P].rearrange(
                                        "p (k j) -> p k j", k=nb
                                    ),
                                    in0=dloc_sb[:, t : t + nb]
                                    .unsqueeze(2)
                                    .to_broadcast([P, nb, P]),
                                    in1=iota_sb[:]
                                    .unsqueeze(1)
                                    .to_broadcast([P, nb, P]),
                                    op=mybir.AluOpType.is_equal,
                                )
                            cp = cpos[s]
                            if cp % K == 0:
                                nbg = min(K, tlc[s] - cp)
                                mt = msgp.tile([P, K * P], BF, tag=f"m{s}")
                                nc.gpsimd.dma_gather(
                                    mt[:, : nbg * P].rearrange(
                                        "p (k f) -> p k f", k=nbg
                                    ),
                                    table16[s * SEGR : (s + 1) * SEGR, :],
                                    idxs_sb[
                                        :,
                                        (coff[s] + cp) * 8 : (coff[s] + cp) * 8
                                        + nbg * 8,
                                    ],
                                    nbg * P,
                                    nbg * P,
                                    P,
                                    single_packet=False,
                                    queue_num=s,
                                )
                                cur_msg[s] = mt
                            done += 1
                            nc.tensor.matmul(
                                out=pw[:],
                                lhsT=s_sb[:, (t % SB) * P : (t % SB + 1) * P],
                                rhs=cur_msg[s][:, (cp % K) * P : (cp % K + 1) * P],
                                start=False,
                                stop=(done == wtot),
                            )
                            cpos[s] += 1
                            t += 1
                    # ---- epilogue -------------------------------------
                    if l < N_LAYERS - 1:
                        # relu and +b commute with the transpose: apply
                        # dis before, b (per-partition post-transpose) after
                        t2 = trp.tile([P, P], BF, tag="t2b")
                        nc.vector.tensor_scalar(
                            out=t2[:],
                            in0=pw[:],
                            scalar1=dis_sb[:, w : w + 1],
                            scalar2=None,
                            op0=mybir.AluOpType.mult,
                        )
                        pt = ptp.tile([P, P], BF, space="PSUM", tag="pt")
                        nc.tensor.transpose(
                            out=pt[:], in_=t2[:], identity=identb_sb[:]
                        )
                        nc.vector.tensor_scalar(
                            out=xt_sb[:, w * P : (w + 1) * P],
                            in0=pt[:],
                            scalar1=bcol_sb[:, l : l + 1],
                            scalar2=0.0,
                            op0=mybir.AluOpType.add,
                            op1=mybir.AluOpType.max,
                        )
                    else:
                        t1 = trp.tile([P, P], F32, tag="t1f")
                        nc.vector.tensor_scalar(
                            out=t1[:],
                            in0=pw[:],
                            scalar1=dis_sb[:, w : w + 1],
                            scalar2=None,
                            op0=mybir.AluOpType.mult,
                        )
                        t2f = trp.tile([P, P], F32, tag="t2f")
                        nc.vector.tensor_tensor(
                            out=t2f[:],
                            in0=t1[:],
                            in1=bbc_sb[:],
                            op=mybir.AluOpType.add,
                        )
                        nc.vector.tensor_scalar(
                            out=t2f[:],
                            in0=t2f[:],
                            scalar1=0.0,
                            scalar2=None,
                            op0=mybir.AluOpType.max,
                        )
                        nc.sync.dma_start(
                            out=out_ap[w * P : (w + 1) * P, :], in_=t2f[:]
                        )

    nc.compile()
    return nc


def build_all(x, edge_indices, W, b):
    in_maps, layout = _prepare(x, edge_indices, W, b)
    nc = _build(layout)
    return nc, in_maps


def kernel(x, edge_indices, W, b):
    nc, in_maps = build_all(x, edge_indices, W, b)
    last_err = None
    for _ in range(3):  # retry transient NRT/axon device faults
        try:
            res = run_bass_kernel_spmd(nc, in_maps, list(range(N_CORES)))
            break
        except Exception as e:  # noqa: BLE001
            last_err = e
            import time as _time

            _time.sleep(5.0)
    else:
        raise last_err
    out = np.concatenate(
        [res.results[c]["out"][:NSH] for c in range(N_CORES)], axis=0
    )
    return out.astype(np.float32)
